# revision 1
# baseline (speedup 1.0000x reference)
"""Trainium2 Bass kernel for the dual-branch agent-attention module.

Sharding: data-parallel over B=8 (one batch element per NeuronCore).
All transposes and weight permutations are done host-side; on-device
work is a streamed bf16 pipeline:

  prep:      agent projections k_ag/qa -> block-diagonal tiles; then
             effective score weights Weff_A = Wq @ k12bd and
             Weff_B = Wkhf @ qabd (associativity: the big activations
             never materialize q or kh at all).
  phase B:   v = attnT^T@Wv (with ones col), tT scores directly from
             attnT via Weff_B -> exp -> xs accumulation (softmax denom
             folded in via the ones column of v_aug).
  phase AC:  sT scores directly from xT via Weff_A -> exp -> PA,
             x_out with ones-column denom, normalize, PE-transpose,
             proj.

Head-major layout trick: the 2C projection outputs are permuted host-
side from (branch, head, d) to (head, branch, d) so each head pair
occupies one 128-partition tile; branch score scales (wa/wb * D^-0.5)
are folded into the K-side weights, so both branches' score maps come
out of a single contraction per head pair.

Bias handling: k-side biases that are constant along a softmax axis
cancel exactly and are dropped (bk_hf entirely; q-side bias of branch
A survives as the per-agent term c_A = k12bd^T @ bq, applied as the
exp's per-partition bias together with ba).
"""

import os
import sys
import numpy as np

for _p in ("/opt/trn_rl_repo", os.path.expanduser("~/.axon_site/_ro/trn_rl_repo")):
    if os.path.isdir(_p) and _p not in sys.path:
        sys.path.insert(0, _p)

import ml_dtypes

import concourse.bass as bass
import concourse.bacc as bacc
import concourse.tile as tile
from concourse import mybir
from concourse.bass_utils import run_bass_kernel_spmd
from concourse.masks import make_identity

BF16 = mybir.dt.bfloat16
F32 = mybir.dt.float32
NPBF16 = ml_dtypes.bfloat16

B, N, NA, H, D = 8, 4096, 64, 12, 32
C = H * D            # 384
C2 = 2 * C           # 768
NP = H // 2          # 6 head pairs
CH = 512             # seq chunk
NCH = N // CH        # 8
TPC = CH // 128      # 4 seq tiles per chunk
SCALE = D ** -0.5

_CACHE = {}


def _build_bass(finalize=True, zero_bias=False):
    nc = bacc.Bacc()

    # ---- DRAM I/O ----
    xT = nc.dram_tensor("xT", [C, N], BF16, kind="ExternalInput")
    attnT = nc.dram_tensor("attnT", [C, N], BF16, kind="ExternalInput")
    agT = nc.dram_tensor("agT", [C, NA], BF16, kind="ExternalInput")
    wqT = nc.dram_tensor("wqT", [C2, C], BF16, kind="ExternalInput")
    wkag = nc.dram_tensor("wkag", [C, C2], BF16, kind="ExternalInput")
    wqag = nc.dram_tensor("wqag", [C, C2], BF16, kind="ExternalInput")
    wkhfT = nc.dram_tensor("wkhfT", [C2, C], BF16, kind="ExternalInput")
    wv = nc.dram_tensor("wv", [C, H * 33], BF16, kind="ExternalInput")
    wproj = nc.dram_tensor("wproj", [C, C], BF16, kind="ExternalInput")
    bq = nc.dram_tensor("bq", [C2], F32, kind="ExternalInput")
    bkag = nc.dram_tensor("bkag", [C2], F32, kind="ExternalInput")
    bqag = nc.dram_tensor("bqag", [C2], F32, kind="ExternalInput")
    bv = nc.dram_tensor("bv", [H * 33], F32, kind="ExternalInput")
    bproj = nc.dram_tensor("bproj", [C], F32, kind="ExternalInput")
    bab = nc.dram_tensor("bab", [2], F32, kind="ExternalInput")
    out = nc.dram_tensor("out", [N, C], F32, kind="ExternalOutput")

    Exp = mybir.ActivationFunctionType.Exp

    def bcast_dram(ap, parts, cols):
        return bass.AP(tensor=ap.tensor, offset=ap.offset, ap=[[0, parts], [1, cols]])

    with tile.TileContext(nc) as tc:
        with (
            tc.tile_pool(name="const", bufs=1) as const,
            tc.tile_pool(name="inp", bufs=4) as p_in,
            tc.tile_pool(name="vv", bufs=2) as p_v,
            tc.tile_pool(name="pt", bufs=4) as p_pt,
            tc.tile_pool(name="pa", bufs=2) as p_pa,
            tc.tile_pool(name="xon", bufs=2) as p_xon,
            tc.tile_pool(name="xot", bufs=4) as p_xot,
            tc.tile_pool(name="osb", bufs=3) as p_out,
            tc.tile_pool(name="sm", bufs=4) as p_sm,
            tc.tile_pool(name="psA", bufs=2, space="PSUM") as psA,
            tc.tile_pool(name="psB", bufs=2, space="PSUM") as psB,
            tc.tile_pool(name="psC", bufs=2, space="PSUM") as psC,
            tc.tile_pool(name="psX", bufs=2, space="PSUM") as psX,
        ):
            # ---- constants ----
            w_qT = const.tile([128, 6, C], BF16)
            w_khfT = const.tile([128, 6, C], BF16)
            w_kag = const.tile([128, 3, C2], BF16)
            w_qag = const.tile([128, 3, C2], BF16)
            w_v = const.tile([128, 3, H * 33], BF16)
            w_pr = const.tile([128, 3, C], BF16)
            for dst, src in ((w_kag, wkag), (w_qag, wqag), (w_v, wv),
                             (w_qT, wqT), (w_khfT, wkhfT)):
                nc.sync.dma_start(out=dst, in_=src.rearrange("(k p) m -> p k m", p=128))
            b_q = const.tile([128, 6], F32)
            b_kag = const.tile([128, 6], F32)
            b_qag = const.tile([128, 6], F32)
            for dst, src in ((b_q, bq), (b_kag, bkag), (b_qag, bqag)):
                nc.gpsimd.dma_start(out=dst, in_=src.rearrange("(j p) -> p j", p=128))
            bv_row = const.tile([1, H * 33], BF16)
            nc.gpsimd.dma_start(out=bv_row, in_=bv[:].unsqueeze(0))
            bpr_row = const.tile([1, C], BF16)
            nc.gpsimd.dma_start(out=bpr_row, in_=bproj[:].unsqueeze(0))
            ones_row = const.tile([1, 128], BF16)
            nc.vector.memset(ones_row, 1.0)
            ones12 = const.tile([1, 12], BF16)
            nc.vector.memset(ones12, 1.0)
            ba_t = const.tile([128, 1], F32)
            nc.gpsimd.dma_start(out=ba_t, in_=bass.AP(tensor=bab[:].tensor, offset=0,
                                                      ap=[[0, 128], [1, 1]]))
            bb_t = const.tile([128, 1], F32)
            nc.gpsimd.dma_start(out=bb_t, in_=bass.AP(tensor=bab[:].tensor, offset=1,
                                                      ap=[[0, 128], [1, 1]]))
            ident = const.tile([128, 128], BF16)
            make_identity(nc, ident)
            ag_t = const.tile([128, 3, NA], BF16)
            nc.gpsimd.dma_start(out=ag_t, in_=agT.rearrange("(k p) m -> p k m", p=128))

            # Pre-touch DMA-loaded constants with tiny reads so wide ops
            # downstream only carry the PE wait.
            touch = const.tile([128, 16], F32)
            for i, t_ap in enumerate((b_q[:, 0:1], b_kag[:, 0:1], b_qag[:, 0:1],
                                      ba_t[:, 0:1], bb_t[:, 0:1])):
                nc.vector.tensor_copy(touch[:, i:i + 1], t_ap)
            nc.scalar.copy(touch[:, 8:9], ba_t[:, 0:1])
            nc.scalar.copy(touch[:, 9:10], bb_t[:, 0:1])

            # ---- prep: k_ag / qa projections -> block-diag tiles ----
            kag_sb = const.tile([128, 6, NA], BF16)
            qa_sb = const.tile([128, 6, NA], BF16)
            for w_t, b_t, dst in ((w_kag, b_kag, kag_sb), (w_qag, b_qag, qa_sb)):
                for j in range(6):
                    ps = psC.tile([128, NA], F32, tag="small")
                    for k in range(3):
                        nc.tensor.matmul(ps, lhsT=w_t[:, k, j * 128:(j + 1) * 128],
                                         rhs=ag_t[:, k, :], start=(k == 0), stop=(k == 2))
                    nc.vector.tensor_add(dst[:, j, :], ps,
                                         b_t[:, j:j + 1].to_broadcast([128, NA]))
            k12bd = const.tile([128, 6, 128], BF16)
            qabd = const.tile([128, 6, 128], BF16)
            for src, dst in ((kag_sb, k12bd), (qa_sb, qabd)):
                nc.vector.memset(dst, 0.0)
                for j in range(6):
                    nc.vector.tensor_copy(dst[0:64, j, 0:64], src[0:64, j, :])
                    nc.vector.tensor_copy(dst[64:128, j, 64:128], src[64:128, j, :])

            # ---- prep: effective score weights + branch-A exp bias ----
            weff_a = const.tile([128, 3, C2], BF16)
            weff_b = const.tile([128, 3, C2], BF16)
            for j in range(6):
                for k in range(3):
                    ps = psC.tile([128, 128], F32, tag="small")
                    nc.tensor.matmul(ps, lhsT=w_qT[:, j, k * 128:(k + 1) * 128],
                                     rhs=k12bd[:, j, :], start=True, stop=True)
                    nc.vector.tensor_copy(weff_a[:, k, j * 128:(j + 1) * 128], ps)
                    ps2 = psC.tile([128, 128], F32, tag="small")
                    nc.tensor.matmul(ps2, lhsT=w_khfT[:, j, k * 128:(k + 1) * 128],
                                     rhs=qabd[:, j, :], start=True, stop=True)
                    nc.scalar.copy(weff_b[:, k, j * 128:(j + 1) * 128], ps2)
            cba = None
            if not zero_bias:
                b_q_bf = const.tile([128, 6], BF16)
                nc.vector.tensor_copy(b_q_bf, b_q)
                cba = const.tile([128, 6], F32)
                for j in range(6):
                    ps = psC.tile([128, 1], F32, tag="small")
                    nc.tensor.matmul(ps, lhsT=k12bd[:, j, :], rhs=b_q_bf[:, j:j + 1],
                                     start=True, stop=True)
                    nc.vector.tensor_add(cba[:, j:j + 1], ps, ba_t[:, 0:1])

            # ---- phase B: values + branch-B attention ----
            xs_sb = const.tile([128, 6 * 66], F32)
            nc.vector.memset(xs_sb, 0.0)
            for c in range(NCH):
                at_t = p_in.tile([128, 3, CH], BF16, tag="inp")
                nc.sync.dma_start(
                    out=at_t,
                    in_=attnT.rearrange("(k p) s -> p k s", p=128)[:, :, c * CH:(c + 1) * CH])
                v_t = p_v.tile([128, TPC, H * 33], BF16)
                for t in range(TPC):
                    ps = psB.tile([128, H * 33], F32, tag="mid")
                    for k in range(3):
                        nc.tensor.matmul(ps, lhsT=at_t[:, k, t * 128:(t + 1) * 128],
                                         rhs=w_v[:, k, :], start=(k == 0), stop=False)
                    if zero_bias:
                        ps33 = ps.rearrange("p (h c) -> p h c", c=33)
                        nc.tensor.matmul(ps33[:, :, 32], lhsT=ones_row[:, :],
                                         rhs=ones12[:, :], start=False, stop=True)
                    else:
                        nc.tensor.matmul(ps, lhsT=ones_row[:, :], rhs=bv_row[:, :],
                                         start=False, stop=True)
                    nc.scalar.copy(v_t[:, t, :], ps)
                for t in range(TPC):
                    xs_stp = psX.tile([128, 6 * 66], F32, tag="xs")
                    ps4 = psC.tile([128, 512], F32, tag="small")
                    ps2 = psC.tile([128, 256], F32, tag="small")
                    for j in range(6):
                        dst = ps4[:, j * 128:(j + 1) * 128] if j < 4 else \
                            ps2[:, (j - 4) * 128:(j - 3) * 128]
                        for k in range(3):
                            nc.tensor.matmul(dst, lhsT=at_t[:, k, t * 128:(t + 1) * 128],
                                             rhs=weff_b[:, k, j * 128:(j + 1) * 128],
                                             start=(k == 0), stop=(k == 2))
                    pt4 = p_pt.tile([128, 512], BF16, tag="pt4")
                    pt2 = p_pt.tile([128, 256], BF16, tag="pt2")
                    bbias = 0.0 if zero_bias else bb_t[:, 0:1]
                    nc.scalar.activation(pt4, ps4, Exp, bias=bbias)
                    nc.scalar.activation(pt2, ps2, Exp, bias=bbias)
                    for j in range(6):
                        lhsT = pt4[:, j * 128:(j + 1) * 128] if j < 4 else \
                            pt2[:, (j - 4) * 128:(j - 3) * 128]
                        nc.tensor.matmul(xs_stp[:, j * 66:(j + 1) * 66], lhsT=lhsT,
                                         rhs=v_t[:, t, j * 66:(j + 1) * 66],
                                         start=True, stop=True)
                    nc.vector.tensor_add(xs_sb, xs_stp, xs_sb)

            nc.sync.dma_start(out=w_pr, in_=wproj.rearrange("(k p) m -> p k m", p=128))

            # ---- xs normalize -> block-diag [xs | 1] tiles ----
            xs_bd = const.tile([128, 6 * 66], BF16)
            xs3 = xs_sb[:].rearrange("p (j c) -> p j c", c=66)
            bd3 = xs_bd[:].rearrange("p (j c) -> p j c", c=66)
            nc.vector.memset(xs_bd, 0.0)
            nc.vector.memset(bd3[0:64, :, 32:33], 1.0)
            nc.vector.memset(bd3[64:128, :, 65:66], 1.0)
            rec6 = p_sm.tile([128, 6], F32, tag="rec")
            nc.vector.reciprocal(rec6[0:64, :], xs3[0:64, :, 32])
            nc.vector.reciprocal(rec6[64:128, :], xs3[64:128, :, 65])
            nc.vector.tensor_mul(bd3[0:64, :, 0:32], xs3[0:64, :, 0:32],
                                 rec6[0:64, :].unsqueeze(2).to_broadcast([64, 6, 32]))
            nc.vector.tensor_mul(bd3[64:128, :, 33:65], xs3[64:128, :, 33:65],
                                 rec6[64:128, :].unsqueeze(2).to_broadcast([64, 6, 32]))

            # ---- phase AC: branch-A attention + proj ----
            for c in range(NCH):
                xt_t = p_in.tile([128, 3, CH], BF16, tag="inp")
                nc.sync.dma_start(
                    out=xt_t,
                    in_=xT.rearrange("(k p) s -> p k s", p=128)[:, :, c * CH:(c + 1) * CH])
                pa_t = p_pa.tile([128, 6, CH], BF16)
                for j in range(6):
                    ps = psA.tile([128, CH], F32, tag="big")
                    for k in range(3):
                        nc.tensor.matmul(ps, lhsT=weff_a[:, k, j * 128:(j + 1) * 128],
                                         rhs=xt_t[:, k, :], start=(k == 0), stop=(k == 2))
                    nc.scalar.activation(pa_t[:, j, :], ps, Exp,
                                         bias=(0.0 if zero_bias else cba[:, j:j + 1]))
                for t in range(TPC):
                    xo_ps = psB.tile([128, 12 * 33], F32, tag="mid")
                    for j in range(6):
                        nc.tensor.matmul(xo_ps[:, j * 66:(j + 1) * 66],
                                         lhsT=pa_t[:, j, t * 128:(t + 1) * 128],
                                         rhs=xs_bd[:, j * 66:(j + 1) * 66],
                                         start=True, stop=True)
                    xo3 = xo_ps.rearrange("p (k c) -> p k c", c=33)
                    rec = p_sm.tile([128, 12], F32, tag="rec12")
                    nc.vector.reciprocal(rec, xo3[:, :, 32])
                    xon = p_xon.tile([128, C], BF16)
                    nc.vector.tensor_mul(xon[:].rearrange("p (k c) -> p k c", c=32),
                                         xo3[:, :, 0:32],
                                         rec[:].unsqueeze(2).to_broadcast([128, 12, 32]))
                    pr_ps = psX.tile([128, C], F32, tag="xs")
                    for f in range(3):
                        tp = psC.tile([128, 128], BF16, tag="small")
                        nc.tensor.transpose(tp, xon[:, f * 128:(f + 1) * 128], ident)
                        xot = p_xot.tile([128, 128], BF16)
                        nc.vector.tensor_copy(xot, tp)
                        nc.tensor.matmul(pr_ps, lhsT=xot, rhs=w_pr[:, f, :],
                                         start=(f == 0),
                                         stop=(zero_bias and f == 2),
                                         skip_group_check=True)
                    if not zero_bias:
                        nc.tensor.matmul(pr_ps, lhsT=ones_row[:, :], rhs=bpr_row[:, :],
                                         start=False, stop=True, skip_group_check=True)
                    o_sb = p_out.tile([128, C], F32)
                    nc.scalar.copy(o_sb, pr_ps)
                    r0 = (c * TPC + t) * 128
                    nc.sync.dma_start(out=out[r0:r0 + 128, :], in_=o_sb)
    if finalize:
        nc.finalize()
    return nc


def _prep_host(inputs):
    f32 = np.float32
    x = np.asarray(inputs["x"], f32)
    attn = np.asarray(inputs["attn"], f32)
    agent = np.asarray(inputs["agent_input"], f32)
    wa = np.asarray(inputs["wa"], f32)
    wb = np.asarray(inputs["wb"], f32)

    perm = np.empty(C2, np.int64)
    sva = np.empty(C2, f32)
    svb = np.empty(C2, f32)
    for h in range(H):
        for br in range(2):
            j0 = h * 64 + br * 32
            perm[j0:j0 + 32] = br * C + h * 32 + np.arange(32)
            sva[j0:j0 + 32] = wa[br] * SCALE
            svb[j0:j0 + 32] = wb[br] * SCALE

    wq_p = np.asarray(inputs["Wq_lf"], f32)[:, perm]
    bq_p = np.asarray(inputs["bq_lf"], f32)[perm]
    wkag_p = np.asarray(inputs["Wk_ag"], f32)[:, perm] * sva[None, :]
    bkag_p = np.asarray(inputs["bk_ag"], f32)[perm] * sva
    wqag_p = np.asarray(inputs["Wq_ag"], f32)[:, perm]
    bqag_p = np.asarray(inputs["bq_ag"], f32)[perm]
    wkhf_p = np.asarray(inputs["Wk_hf"], f32)[:, perm] * svb[None, :]

    wv_in = np.asarray(inputs["Wv_hf"], f32)
    bv_in = np.asarray(inputs["bv_hf"], f32)
    wv_aug = np.zeros((C, H * 33), f32)
    bv_aug = np.zeros(H * 33, f32)
    for h in range(H):
        wv_aug[:, h * 33:h * 33 + 32] = wv_in[:, h * 32:h * 32 + 32]
        bv_aug[h * 33:h * 33 + 32] = bv_in[h * 32:h * 32 + 32]
        bv_aug[h * 33 + 32] = 1.0

    bab = np.array([np.asarray(inputs["ba"], f32)[0],
                    np.asarray(inputs["bb"], f32)[0]], f32)

    shared = {
        "wqT": np.ascontiguousarray(wq_p.T).astype(NPBF16),
        "wkhfT": np.ascontiguousarray(wkhf_p.T).astype(NPBF16),
        "wkag": wkag_p.astype(NPBF16),
        "wqag": wqag_p.astype(NPBF16),
        "wv": wv_aug.astype(NPBF16),
        "wproj": np.asarray(inputs["Wproj"], f32).astype(NPBF16),
        "bq": bq_p, "bkag": bkag_p, "bqag": bqag_p,
        "bv": bv_aug, "bproj": np.ascontiguousarray(np.asarray(inputs["bproj"], f32)),
        "bab": bab,
    }
    xT = np.ascontiguousarray(x.transpose(0, 2, 1)).astype(NPBF16)
    attnT = np.ascontiguousarray(attn.transpose(0, 2, 1)).astype(NPBF16)
    agT = np.ascontiguousarray(agent.transpose(0, 2, 1)).astype(NPBF16)
    in_maps = []
    for b in range(B):
        m = dict(shared)
        m["xT"] = xT[b]
        m["attnT"] = attnT[b]
        m["agT"] = agT[b]
        in_maps.append(m)
    return in_maps


def kernel(**inputs):
    zb = all(not np.any(np.asarray(inputs[k]))
             for k in ("bq_lf", "bk_ag", "bq_ag", "bk_hf", "bv_hf", "bproj",
                       "ba", "bb"))
    key = ("nc", zb)
    if key not in _CACHE:
        _CACHE[key] = _build_bass(zero_bias=zb)
    nc = _CACHE[key]
    in_maps = _prep_host(inputs)
    res = run_bass_kernel_spmd(nc, in_maps, core_ids=list(range(B)))
    return np.stack([res.results[b]["out"] for b in range(B)], axis=0)



# revision 2
# speedup vs baseline: 1.1889x; 1.1889x over previous
"""Trainium2 Bass kernel for the dual-branch agent-attention module.

Sharding: data-parallel over B=8 (one batch element per NeuronCore).
All transposes and weight permutations are done host-side; on-device
work is a streamed bf16 pipeline.

Math restructuring vs the reference:
  - Effective score weights Weff_A = Wq @ k12bd and Weff_B = Wkhf @ qabd
    (associativity: the big activations never materialize q or kh).
  - Scalar softmax biases ba/bb cancel (softmax shift invariance) and
    are dropped. Branch-A's per-agent bias c_A = k12bd^T @ bq survives
    as the exp's per-partition bias.
  - v bias bv is folded in AFTER the xs softmax-normalize
    (xs_n = xs0/denom + bv), so the per-tile v-bias matmul disappears;
    the softmax denominators come from ones columns memset into the
    v tile.
  - proj bias is added host-side.

Dataflow (per core):
  stage 1 (per 512-col chunk of N):
    B: v = attnT^T@Wv, scores t = attnT^T@Weff_B (wide 512/256 rhs,
       stationary operand shared k-major), exp on ACT straight from
       PSUM, xs accumulated in PSUM across all 32 seq tiles (a single
       K=1 zeroing matmul opens the accumulation region).
    AC: scores s = Weff_A^T@xT per head pair, exp(+c_A) into a
       persistent SBUF pa buffer.
  stage 1.5: xs normalize -> block-diag [xs | 1] tiles.
  stage 2 (per seq tile): x_out = PA^T @ xs_bd with ones-column
    denominators, normalize, PE-transpose, proj, store.
"""

import os
import sys
import numpy as np

for _p in ("/opt/trn_rl_repo", os.path.expanduser("~/.axon_site/_ro/trn_rl_repo")):
    if os.path.isdir(_p) and _p not in sys.path:
        sys.path.insert(0, _p)

import ml_dtypes

import concourse.bass as bass
import concourse.bacc as bacc
import concourse.tile as tile
from concourse import mybir
from concourse.bass_utils import run_bass_kernel_spmd
from concourse.masks import make_identity

BF16 = mybir.dt.bfloat16
F32 = mybir.dt.float32
NPBF16 = ml_dtypes.bfloat16

B, N, NA, H, D = 8, 4096, 64, 12, 32
C = H * D            # 384
C2 = 2 * C           # 768
NP = H // 2          # 6 head pairs
CH = 512             # seq chunk
NCH = N // CH        # 8
TPC = CH // 128      # 4 seq tiles per chunk
SCALE = D ** -0.5

_CACHE = {}


def _build_bass(finalize=True, zero_bias=False):
    nc = bacc.Bacc()

    # ---- DRAM I/O ----
    xT = nc.dram_tensor("xT", [C, N], BF16, kind="ExternalInput")
    attnT = nc.dram_tensor("attnT", [C, N], BF16, kind="ExternalInput")
    agT = nc.dram_tensor("agT", [C, NA], BF16, kind="ExternalInput")
    wqT = nc.dram_tensor("wqT", [C2, C], BF16, kind="ExternalInput")
    wkag = nc.dram_tensor("wkag", [C, C2], BF16, kind="ExternalInput")
    wqag = nc.dram_tensor("wqag", [C, C2], BF16, kind="ExternalInput")
    wkhfT = nc.dram_tensor("wkhfT", [C2, C], BF16, kind="ExternalInput")
    wv = nc.dram_tensor("wv", [C, C], BF16, kind="ExternalInput")
    wproj = nc.dram_tensor("wproj", [C, C], BF16, kind="ExternalInput")
    if not zero_bias:
        bq = nc.dram_tensor("bq", [C2], F32, kind="ExternalInput")
        bkag = nc.dram_tensor("bkag", [C2], F32, kind="ExternalInput")
        bqag = nc.dram_tensor("bqag", [C2], F32, kind="ExternalInput")
        bvh = nc.dram_tensor("bvh", [2 * NP * D], F32, kind="ExternalInput")
    out = nc.dram_tensor("out", [N, C], F32, kind="ExternalOutput")

    Exp = mybir.ActivationFunctionType.Exp

    with tile.TileContext(nc) as tc:
        with (
            tc.tile_pool(name="const", bufs=1) as const,
            tc.tile_pool(name="vv", bufs=2) as p_v,
            tc.tile_pool(name="pt", bufs=3) as p_pt,
            tc.tile_pool(name="xon", bufs=2) as p_xon,
            tc.tile_pool(name="xot", bufs=3) as p_xot,
            tc.tile_pool(name="osb", bufs=3) as p_out,
            tc.tile_pool(name="sm", bufs=4) as p_sm,
            tc.tile_pool(name="psA", bufs=3, space="PSUM") as psA,
            tc.tile_pool(name="psC", bufs=2, space="PSUM") as psC,
            tc.tile_pool(name="psT", bufs=2, space="PSUM") as psT,
            tc.tile_pool(name="psX", bufs=1, space="PSUM") as psX,
        ):
            # ---- constants ----
            w_qT = const.tile([128, 6, C], BF16)
            w_khfT = const.tile([128, 6, C], BF16)
            w_kag = const.tile([128, 3, C2], BF16)
            w_qag = const.tile([128, 3, C2], BF16)
            w_v = const.tile([128, 3, C], BF16)
            w_pr = const.tile([128, 3, C], BF16)
            for dst, src in ((w_kag, wkag), (w_qag, wqag), (w_v, wv),
                             (w_qT, wqT), (w_khfT, wkhfT), (w_pr, wproj)):
                nc.sync.dma_start(out=dst, in_=src.rearrange("(k p) m -> p k m", p=128))
            ag_t = const.tile([128, 3, NA], BF16)
            nc.gpsimd.dma_start(out=ag_t, in_=agT.rearrange("(k p) m -> p k m", p=128))
            if not zero_bias:
                b_q = const.tile([128, 6], F32)
                b_kag = const.tile([128, 6], F32)
                b_qag = const.tile([128, 6], F32)
                for dst, src in ((b_q, bq), (b_kag, bkag), (b_qag, bqag)):
                    nc.gpsimd.dma_start(out=dst, in_=src.rearrange("(j p) -> p j", p=128))
                bvb = const.tile([128, NP, D], F32)
                nc.gpsimd.dma_start(
                    out=bvb[0:64],
                    in_=bass.AP(tensor=bvh[:].tensor, offset=0,
                                ap=[[0, 64], [1, NP * D]]))
                nc.gpsimd.dma_start(
                    out=bvb[64:128],
                    in_=bass.AP(tensor=bvh[:].tensor, offset=NP * D,
                                ap=[[0, 64], [1, NP * D]]))
            ident = const.tile([128, 128], BF16)
            make_identity(nc, ident)
            zrow = const.tile([1, 396], BF16)
            nc.vector.memset(zrow, 0.0)

            # full activations resident in SBUF
            at_full = const.tile([128, 3, N], BF16)
            xt_full = const.tile([128, 3, N], BF16)
            for c in range(NCH):
                nc.sync.dma_start(
                    out=at_full[:, :, c * CH:(c + 1) * CH],
                    in_=attnT.rearrange("(k p) s -> p k s", p=128)[:, :, c * CH:(c + 1) * CH])
                nc.sync.dma_start(
                    out=xt_full[:, :, c * CH:(c + 1) * CH],
                    in_=xT.rearrange("(k p) s -> p k s", p=128)[:, :, c * CH:(c + 1) * CH])
            pa_full = const.tile([128, 6, N], BF16)

            # Pre-touch DMA-loaded bias constants with tiny reads.
            if not zero_bias:
                touch = const.tile([128, 16], F32)
                for i, t_ap in enumerate((b_q[:, 0:1], b_kag[:, 0:1],
                                          b_qag[:, 0:1], bvb[:, 0:1, 0])):
                    nc.vector.tensor_copy(touch[:, i:i + 1], t_ap)

            # ---- prep: k_ag / qa projections -> block-diag tiles ----
            kag_sb = const.tile([128, 6, NA], BF16)
            qa_sb = const.tile([128, 6, NA], BF16)
            for w_t, b_t, dst in ((w_kag, "bkag", kag_sb), (w_qag, "bqag", qa_sb)):
                for j in range(6):
                    ps = psA.tile([128, NA], F32, tag="pA")
                    for k in range(3):
                        nc.tensor.matmul(ps, lhsT=w_t[:, k, j * 128:(j + 1) * 128],
                                         rhs=ag_t[:, k, :], start=(k == 0), stop=(k == 2))
                    if zero_bias:
                        nc.vector.tensor_copy(dst[:, j, :], ps)
                    else:
                        bt = b_kag if b_t == "bkag" else b_qag
                        nc.vector.tensor_add(dst[:, j, :], ps,
                                             bt[:, j:j + 1].to_broadcast([128, NA]))
            k12bd = const.tile([128, 6, 128], BF16)
            qabd = const.tile([128, 6, 128], BF16)
            for src, dst in ((kag_sb, k12bd), (qa_sb, qabd)):
                nc.vector.memset(dst, 0.0)
                for j in range(6):
                    nc.vector.tensor_copy(dst[0:64, j, 0:64], src[0:64, j, :])
                    nc.vector.tensor_copy(dst[64:128, j, 64:128], src[64:128, j, :])

            # ---- prep: effective score weights + branch-A exp bias ----
            weff_a = const.tile([128, 3, C2], BF16)
            weff_b = const.tile([128, 3, C2], BF16)
            for j in range(6):
                for k in range(3):
                    ps = psA.tile([128, 128], F32, tag="pA")
                    nc.tensor.matmul(ps, lhsT=w_qT[:, j, k * 128:(k + 1) * 128],
                                     rhs=k12bd[:, j, :], start=True, stop=True)
                    nc.vector.tensor_copy(weff_a[:, k, j * 128:(j + 1) * 128], ps)
                    ps2 = psA.tile([128, 128], F32, tag="pA")
                    nc.tensor.matmul(ps2, lhsT=w_khfT[:, j, k * 128:(k + 1) * 128],
                                     rhs=qabd[:, j, :], start=True, stop=True)
                    nc.scalar.copy(weff_b[:, k, j * 128:(j + 1) * 128], ps2)
            cba = None
            if not zero_bias:
                b_q_bf = const.tile([128, 6], BF16)
                nc.vector.tensor_copy(b_q_bf, b_q)
                cba = const.tile([128, 6], F32)
                for j in range(6):
                    ps = psA.tile([128, 1], F32, tag="pA")
                    nc.tensor.matmul(ps, lhsT=k12bd[:, j, :], rhs=b_q_bf[:, j:j + 1],
                                     start=True, stop=True)
                    nc.vector.tensor_copy(cba[:, j:j + 1], ps)

            # ---- xs accumulator: open the PSUM region with a zero matmul ----
            xs_acc = psX.tile([128, 6, 66], F32)
            nc.tensor.matmul(xs_acc[:, :, :], lhsT=zrow[:, 0:128], rhs=zrow[:, 0:396],
                             start=True, stop=False, skip_group_check=True)

            # ---- stage 1: values + branch-B attention + branch-A scores ----
            pending_xs = None
            for c in range(NCH):
                v_t = p_v.tile([128, TPC, H, 33], BF16)
                nc.vector.memset(v_t[:, :, :, 32], 1.0)
                for t in range(TPC):
                    s0 = c * CH + t * 128
                    psv = psC.tile([128, C], F32, tag="pC")
                    ps4 = psA.tile([128, 512], F32, tag="pA")
                    ps2 = psC.tile([128, 256], F32, tag="pC")
                    for k in range(3):
                        at_k = at_full[:, k, s0:s0 + 128]
                        nc.tensor.matmul(psv, lhsT=at_k, rhs=w_v[:, k, :],
                                         start=(k == 0), stop=(k == 2))
                        nc.tensor.matmul(ps4, lhsT=at_k, rhs=weff_b[:, k, 0:512],
                                         start=(k == 0), stop=(k == 2))
                        nc.tensor.matmul(ps2, lhsT=at_k, rhs=weff_b[:, k, 512:768],
                                         start=(k == 0), stop=(k == 2))
                    pt = p_pt.tile([128, 768], BF16)
                    nc.scalar.activation(pt[:, 0:512], ps4, Exp)
                    nc.scalar.activation(pt[:, 512:768], ps2, Exp)
                    nc.vector.tensor_copy(
                        v_t[:, t, :, 0:32],
                        psv[:].rearrange("p (h d) -> p h d", d=32))
                    if pending_xs is not None:
                        pending_xs()
                    last = (c == NCH - 1 and t == TPC - 1)

                    def make_xs(pt=pt, v_t=v_t, t=t, last=last):
                        def emit():
                            for j in range(6):
                                nc.tensor.matmul(
                                    xs_acc[:, j, :], lhsT=pt[:, j * 128:(j + 1) * 128],
                                    rhs=v_t[:, t, 2 * j:2 * j + 2, :],
                                    start=False, stop=(last and j == 5),
                                    skip_group_check=True)
                        return emit
                    pending_xs = make_xs()
                for j in range(6):
                    ps = psA.tile([128, CH], F32, tag="pA")
                    for k in range(3):
                        nc.tensor.matmul(ps, lhsT=weff_a[:, k, j * 128:(j + 1) * 128],
                                         rhs=xt_full[:, k, c * CH:(c + 1) * CH],
                                         start=(k == 0), stop=(k == 2))
                    nc.scalar.activation(
                        pa_full[:, j, c * CH:(c + 1) * CH], ps, Exp,
                        bias=(0.0 if zero_bias else cba[:, j:j + 1]))
            pending_xs()

            # ---- stage 1.5: xs normalize -> block-diag [xs | 1] tiles ----
            xs_bd = const.tile([128, 6, 66], BF16)
            nc.vector.memset(xs_bd, 0.0)
            nc.vector.memset(xs_bd[0:64, :, 32:33], 1.0)
            nc.vector.memset(xs_bd[64:128, :, 65:66], 1.0)
            rec6 = p_sm.tile([128, 6], F32, tag="rec")
            nc.vector.reciprocal(rec6[0:64, :], xs_acc[0:64, :, 32])
            nc.vector.reciprocal(rec6[64:128, :], xs_acc[64:128, :, 65])
            nc.vector.tensor_mul(xs_bd[0:64, :, 0:32], xs_acc[0:64, :, 0:32],
                                 rec6[0:64, :].unsqueeze(2).to_broadcast([64, 6, 32]))
            nc.vector.tensor_mul(xs_bd[64:128, :, 33:65], xs_acc[64:128, :, 33:65],
                                 rec6[64:128, :].unsqueeze(2).to_broadcast([64, 6, 32]))
            if not zero_bias:
                nc.vector.tensor_add(xs_bd[0:64, :, 0:32], xs_bd[0:64, :, 0:32],
                                     bvb[0:64])
                nc.vector.tensor_add(xs_bd[64:128, :, 33:65], xs_bd[64:128, :, 33:65],
                                     bvb[64:128])

            # ---- stage 2: branch-A attention + proj ----
            for c in range(NCH):
                for t in range(TPC):
                    s0 = c * CH + t * 128
                    xo = psC.tile([128, 396], F32, tag="pC")
                    for j in range(6):
                        nc.tensor.matmul(xo[:, j * 66:(j + 1) * 66],
                                         lhsT=pa_full[:, j, s0:s0 + 128],
                                         rhs=xs_bd[:, j, :],
                                         start=True, stop=True)
                    xo3 = xo[:].rearrange("p (h d) -> p h d", d=33)
                    rec = p_sm.tile([128, 12], F32, tag="rec12")
                    nc.vector.reciprocal(rec, xo3[:, :, 32])
                    xon = p_xon.tile([128, C], BF16)
                    nc.vector.tensor_mul(xon[:].rearrange("p (h d) -> p h d", d=32),
                                         xo3[:, :, 0:32],
                                         rec[:].unsqueeze(2).to_broadcast([128, 12, 32]))
                    pr = psA.tile([128, C], F32, tag="pA")
                    for f in range(3):
                        tp = psT.tile([128, 128], BF16)
                        nc.tensor.transpose(tp, xon[:, f * 128:(f + 1) * 128], ident)
                        xot = p_xot.tile([128, 128], BF16)
                        if f == 0:
                            nc.vector.tensor_copy(xot, tp)
                        else:
                            nc.scalar.copy(xot, tp)
                        nc.tensor.matmul(pr, lhsT=xot, rhs=w_pr[:, f, :],
                                         start=(f == 0), stop=(f == 2),
                                         skip_group_check=True)
                    o_sb = p_out.tile([128, C], F32)
                    nc.scalar.copy(o_sb, pr)
                    nc.sync.dma_start(out=out[s0:s0 + 128, :], in_=o_sb)
    if finalize:
        nc.finalize()
    return nc


def _prep_host(inputs):
    f32 = np.float32
    x = np.asarray(inputs["x"], f32)
    attn = np.asarray(inputs["attn"], f32)
    agent = np.asarray(inputs["agent_input"], f32)
    wa = np.asarray(inputs["wa"], f32)
    wb = np.asarray(inputs["wb"], f32)

    perm = np.empty(C2, np.int64)
    sva = np.empty(C2, f32)
    svb = np.empty(C2, f32)
    for h in range(H):
        for br in range(2):
            j0 = h * 64 + br * 32
            perm[j0:j0 + 32] = br * C + h * 32 + np.arange(32)
            sva[j0:j0 + 32] = wa[br] * SCALE
            svb[j0:j0 + 32] = wb[br] * SCALE

    wq_p = np.asarray(inputs["Wq_lf"], f32)[:, perm]
    bq_p = np.asarray(inputs["bq_lf"], f32)[perm]
    wkag_p = np.asarray(inputs["Wk_ag"], f32)[:, perm] * sva[None, :]
    bkag_p = np.asarray(inputs["bk_ag"], f32)[perm] * sva
    wqag_p = np.asarray(inputs["Wq_ag"], f32)[:, perm]
    bqag_p = np.asarray(inputs["bq_ag"], f32)[perm]
    wkhf_p = np.asarray(inputs["Wk_hf"], f32)[:, perm] * svb[None, :]

    zb = all(not np.any(np.asarray(inputs[k]))
             for k in ("bq_lf", "bk_ag", "bq_ag", "bk_hf", "bv_hf", "ba", "bb"))

    shared = {
        "wqT": np.ascontiguousarray(wq_p.T).astype(NPBF16),
        "wkhfT": np.ascontiguousarray(wkhf_p.T).astype(NPBF16),
        "wkag": wkag_p.astype(NPBF16),
        "wqag": wqag_p.astype(NPBF16),
        "wv": np.asarray(inputs["Wv_hf"], f32).astype(NPBF16),
        "wproj": np.asarray(inputs["Wproj"], f32).astype(NPBF16),
    }
    if not zb:
        bv_in = np.asarray(inputs["bv_hf"], f32)
        # bvh[half, j, d]: half 0 = head 2j, half 1 = head 2j+1
        bvh = np.empty((2, NP, D), f32)
        for j in range(NP):
            bvh[0, j, :] = bv_in[(2 * j) * D:(2 * j + 1) * D]
            bvh[1, j, :] = bv_in[(2 * j + 1) * D:(2 * j + 2) * D]
        shared.update({
            "bq": bq_p, "bkag": bkag_p, "bqag": bqag_p,
            "bvh": np.ascontiguousarray(bvh.reshape(-1)),
        })
    xT = np.ascontiguousarray(x.transpose(0, 2, 1)).astype(NPBF16)
    attnT = np.ascontiguousarray(attn.transpose(0, 2, 1)).astype(NPBF16)
    agT = np.ascontiguousarray(agent.transpose(0, 2, 1)).astype(NPBF16)
    in_maps = []
    for b in range(B):
        m = dict(shared)
        m["xT"] = xT[b]
        m["attnT"] = attnT[b]
        m["agT"] = agT[b]
        in_maps.append(m)
    return in_maps, zb


def kernel(**inputs):
    in_maps, zb = _prep_host(inputs)
    key = ("nc", zb)
    if key not in _CACHE:
        _CACHE[key] = _build_bass(zero_bias=zb)
    nc = _CACHE[key]
    res = run_bass_kernel_spmd(nc, in_maps, core_ids=list(range(B)))
    outs = np.stack([res.results[b]["out"] for b in range(B)], axis=0)
    if not zb:
        outs = outs + np.asarray(inputs["bproj"], np.float32)[None, None, :]
    return outs
